# revision 19
# baseline (speedup 1.0000x reference)
"""Trainium2 Bass kernel: dense attention with key-padding mask (ColoAttention).

Math (per batch b, head h):
    scores = (Q @ K^T) / sqrt(D); masked keys -> -inf; softmax over keys;
    out = probs @ V; rows at masked query positions zeroed.

Implementation notes:
  - K and V rows at masked key positions are zeroed on the host.  Then
    scores at masked keys are exactly 0, exp(0) = 1, and the per-row sum of
    exponentials just needs the (host-known) masked-key count subtracted.
    Masked keys contribute 0 to probs @ V since their V rows are zero.
  - Scores are computed transposed (S^T[k, q] = K @ Q^T) so the exp output
    P^T[k, q] (bf16) is directly the moving operand for O'^T = V^T @ P^T.
  - Row sums come from a ones-vector matmul over P^T; the softmax division
    and query masking are folded into one per-partition scale applied while
    copying the (PE-transposed) output out of PSUM.
  - QK^T runs in float32r (full-rate fp32 path on the PE), PV in bf16.
  - Sharding: B*H = 64 (b,h) pairs; core c handles batch c//2, heads
    (c%2)*8 .. +8.  Pure SPMD, no collectives.
"""

import numpy as np
import ml_dtypes
from contextlib import ExitStack

import concourse.bass as bass
import concourse.mybir as mybir
import concourse.tile as tile
from concourse import bacc
from concourse.bass_utils import run_bass_kernel_spmd
from concourse.masks import make_identity

B, S, H, D = 4, 2048, 16, 128
N_CORES = 8
CORES_PER_BATCH = N_CORES // B            # 2
HPC = H // CORES_PER_BATCH                # 8 heads (pairs) per core
P = 128
SCALE = 1.0 / float(np.sqrt(np.float64(D)).astype(np.float32))


def build_program(n_pairs: int = HPC, seq: int = S) -> bacc.Bacc:
    KCN = seq // P          # key chunks of 128
    NW = seq // 512         # 512-wide q windows
    QTN = seq // P          # q tiles of 128
    f32 = mybir.dt.float32
    f32r = mybir.dt.float32r
    bf16 = mybir.dt.bfloat16
    Exp = mybir.ActivationFunctionType.Exp

    nc = bacc.Bacc("TRN2", target_bir_lowering=False, debug=False)
    # q/k arrive pre-transposed from the host: [pair, D, S]
    q_d = nc.dram_tensor("q", [n_pairs, P, seq], f32r, kind="ExternalInput").ap()
    k_d = nc.dram_tensor("k", [n_pairs, P, seq], f32r, kind="ExternalInput").ap()
    v_d = nc.dram_tensor("v", [n_pairs, seq, P], bf16, kind="ExternalInput").ap()
    qmask_d = nc.dram_tensor("qmask", [seq], f32, kind="ExternalInput").ap()
    mcnt_d = nc.dram_tensor("mcount", [P, 1], f32, kind="ExternalInput").ap()
    out_d = nc.dram_tensor("out", [n_pairs, seq, P], f32, kind="ExternalOutput").ap()

    # q is processed in halves of QH columns; scores PSUM double-buffered so
    # QK(kc+1) overlaps exp(kc); ones-matmul row sums interleave per kc.
    QH = min(seq, 1024)      # q-half width
    NH = seq // QH           # number of halves
    NWH = QH // 512          # 512-wide windows per half
    QTH = QH // P            # 128-wide q tiles per half

    with tile.TileContext(nc) as tc:
        with ExitStack() as ctx:
            consts = ctx.enter_context(tc.tile_pool(name="consts", bufs=1))
            inp = ctx.enter_context(tc.tile_pool(name="inp", bufs=2))
            qtp = ctx.enter_context(tc.tile_pool(name="qtp", bufs=2))
            ptp = ctx.enter_context(tc.tile_pool(name="ptp", bufs=2))
            otp = ctx.enter_context(tc.tile_pool(name="otp", bufs=4))
            outp = ctx.enter_context(tc.tile_pool(name="outp", bufs=4))
            smp = ctx.enter_context(tc.tile_pool(name="smp", bufs=2))
            pqp = ctx.enter_context(tc.tile_pool(name="pqp", bufs=2))
            # PSUM (8 banks): "s" scores [128,QH] x3 bufs = 6 banks,
            # "o" O' accum [128,512] x2 = 2 banks (also O transposes).
            sps = ctx.enter_context(tc.tile_pool(name="sps", bufs=3, space="PSUM"))
            ops = ctx.enter_context(tc.tile_pool(name="ops", bufs=2, space="PSUM"))

            ident32 = consts.tile([P, P], f32, tag="ident32")
            make_identity(nc, ident32[:])
            qmaskT = consts.tile([P, QTN], f32, tag="qmaskT")
            nc.sync.dma_start(qmaskT[:], qmask_d.rearrange("(t r) -> r t", r=P))
            mcnt = consts.tile([P, 1], f32, tag="mcnt")
            nc.sync.dma_start(mcnt[:], mcnt_d[:])

            # Tile's dependency tracker does not model InstDmaTransposeAnt
            # reads/writes; wire the edges around it explicitly.
            xbars = {}     # global half index -> [xbar insts per kc]
            reduces = {}   # global half index -> [reduce insts]
            PQ_BUFS = 2
            PT_BUFS = 2

            for p in range(n_pairs):
                # ---- stage inputs (q/k already transposed on host) ----
                qt_sb = qtp.tile([P, seq], f32r, tag="qt")
                nc.sync.dma_start(qt_sb[:], q_d[p])
                kt_sb = qtp.tile([P, seq], f32r, tag="kt")
                nc.sync.dma_start(kt_sb[:], k_d[p])
                v_sb = inp.tile([P, KCN, P], bf16, tag="v_sb")
                nc.sync.dma_start(v_sb[:], v_d[p].rearrange("(t r) d -> r t d", r=P))

                # ---- scores + exp + PV + row sums, software-pipelined so
                # PE always has QK(step+1) to run while ACT does exp(step).
                def emit_qk(h, kc, tag_i):
                    s_ps = sps.tile([P, QH], f32, tag="s", name=f"s_{p}_{h}_{kc}")
                    for w in range(NWH):
                        nc.tensor.matmul(
                            s_ps[:, w * 512:(w + 1) * 512],
                            lhsT=kt_sb[:, kc * P:(kc + 1) * P],
                            rhs=qt_sb[:, h * QH + w * 512:h * QH + (w + 1) * 512],
                            start=True, stop=True)
                    return s_ps

                steps = [(h, kc) for h in range(NH) for kc in range(KCN)]
                pt_half = {}
                ot_half = {}
                pq_half = {}
                pend = {j: emit_qk(*steps[j], j) for j in range(min(2, len(steps)))}
                for i, (h, kc) in enumerate(steps):
                    q0 = h * QH
                    if h not in pt_half:
                        pt_half[h] = ptp.tile([P, KCN, QH], bf16, tag="pt",
                                              name=f"pt_{p}_{h}")
                        ot_half[h] = [
                            ops.tile([P, 512], f32, tag="o", name=f"ot_{p}_{h}_{w}")
                            for w in range(NWH)]
                        pq_half[h] = pqp.tile([P, QTH, KCN * P], bf16, tag="pq",
                                              name=f"pq_{p}_{h}")
                    pt_sb, ot_ps, pq_sb = pt_half[h], ot_half[h], pq_half[h]
                    s_ps = pend.pop(i)
                    ex = nc.scalar.activation(
                        pt_sb[:, kc, :], s_ps[:], Exp, scale=SCALE)
                    gx = p * NH + h
                    if gx - PT_BUFS in xbars:
                        bass._add_dep_helper(
                            ex.ins, xbars[gx - PT_BUFS][kc], sync=True,
                            reason="exp WAR on prior pt xbar read")
                    if i + 2 < len(steps):
                        pend[i + 2] = emit_qk(*steps[i + 2], i + 2)
                    # block-transpose P^T[kc] so row sums become a free-axis
                    # reduction: pq[:, qt, kc*P:(kc+1)*P] = P[q-block qt, kc]
                    g = p * NH + h
                    xb = nc.scalar.dma_start(
                        pq_sb[:, :, kc * P:(kc + 1) * P], pt_sb[:, kc, :],
                        transpose=True)
                    xbars.setdefault(g, []).append(xb.ins)
                    if g - PQ_BUFS in reduces:
                        for rd in reduces[g - PQ_BUFS]:
                            bass._add_dep_helper(
                                xb.ins, rd, sync=True,
                                reason="xbar WAR on prior pq readers")
                    for w in range(NWH):
                        nc.tensor.matmul(
                            ot_ps[w][:],
                            lhsT=v_sb[:, kc, :],
                            rhs=pt_sb[:, kc, w * 512:(w + 1) * 512],
                            start=(kc == 0), stop=(kc == KCN - 1))
                    if kc != KCN - 1:
                        continue

                    # ---- half tail: stage O', sums, scale, store ----
                    otsb = []
                    for w in range(NWH):
                        o_stage = otp.tile([P, 512], f32, tag="otsb",
                                           name=f"otsb_{p}_{h}_{w}")
                        nc.vector.tensor_copy(out=o_stage[:], in_=ot_ps[w][:])
                        otsb.append(o_stage)
                    sumsT = smp.tile([P, QTH], f32, tag="sumsT")
                    for qt in range(QTH):
                        rd = nc.vector.reduce_sum(
                            sumsT[:, qt:qt + 1], pq_sb[:, qt, :],
                            axis=mybir.AxisListType.X)
                        reduces.setdefault(g, []).append(rd.ins)
                        for xb_ins in xbars[g]:
                            bass._add_dep_helper(
                                rd.ins, xb_ins, sync=True,
                                reason="reduce RAW on xbar writes")
                    scaleT = smp.tile([P, QTH], f32, tag="scaleT")
                    nc.vector.tensor_scalar_sub(scaleT[:], sumsT[:], mcnt[:, 0:1])
                    nc.vector.reciprocal(scaleT[:], scaleT[:])
                    nc.vector.tensor_tensor(
                        scaleT[:], scaleT[:],
                        qmaskT[:, h * QTH:(h + 1) * QTH], mybir.AluOpType.mult)

                    for w in range(NWH):
                        for j in range(4):
                            qt = w * 4 + j
                            otr = ops.tile([P, P], f32, tag="o")
                            nc.tensor.transpose(
                                otr[:], otsb[w][:, j * P:(j + 1) * P], ident32[:])
                            o_sb = outp.tile([P, P], f32, tag="o_sb")
                            nc.vector.tensor_scalar_mul(
                                o_sb[:], otr[:], scaleT[:, qt:qt + 1])
                            gq = q0 + qt * P
                            nc.sync.dma_start(out_d[p, gq:gq + P, :], o_sb[:])

    nc.compile()
    return nc


_PROG_CACHE: dict = {}


def _get_program() -> bacc.Bacc:
    if "nc" not in _PROG_CACHE:
        _PROG_CACHE["nc"] = build_program(HPC, S)
    return _PROG_CACHE["nc"]


def make_in_maps(query, key, value, attn_mask):
    # device wants q/k as [pair, D, S] (pre-transposed), v as [pair, S, D]
    qT = np.ascontiguousarray(np.asarray(query, np.float32).transpose(0, 2, 3, 1))
    kT = np.asarray(key, np.float32).transpose(0, 2, 3, 1)       # [B, H, D, S]
    v = np.asarray(value, np.float32).transpose(0, 2, 1, 3)      # [B, H, S, D]
    mf = (np.asarray(attn_mask) > 0).astype(np.float32)          # [B, S]
    kTz = np.ascontiguousarray(kT * mf[:, None, None, :])
    vz = (v * mf[:, None, :, None]).astype(ml_dtypes.bfloat16)
    mcount = (S - mf.sum(axis=1)).astype(np.float32)             # [B]
    in_maps = []
    for c in range(N_CORES):
        b, h0 = c // CORES_PER_BATCH, (c % CORES_PER_BATCH) * HPC
        in_maps.append({
            "q": np.ascontiguousarray(qT[b, h0:h0 + HPC]),
            "k": np.ascontiguousarray(kTz[b, h0:h0 + HPC]),
            "v": np.ascontiguousarray(vz[b, h0:h0 + HPC]),
            "qmask": mf[b],
            "mcount": np.full((P, 1), mcount[b], np.float32),
        })
    return in_maps, mf


def assemble_output(results, mf):
    out = np.empty((B, S, H * D), np.float32)
    for c in range(N_CORES):
        b, h0 = c // CORES_PER_BATCH, (c % CORES_PER_BATCH) * HPC
        o = results[c]["out"]                                    # [HPC, S, D]
        for i in range(HPC):
            out[b, :, (h0 + i) * D:(h0 + i + 1) * D] = o[i]
    for b in range(B):
        if mf[b].sum() == 0.0:                                   # degenerate batch
            out[b] = 0.0
    return out


def kernel(query, key, value, attn_mask):
    nc = _get_program()
    in_maps, mf = make_in_maps(query, key, value, attn_mask)
    res = run_bass_kernel_spmd(nc, in_maps, list(range(N_CORES)))
    return assemble_output(res.results, mf)


# revision 20
# speedup vs baseline: 1.8415x; 1.8415x over previous
"""Trainium2 Bass kernel: dense attention with key-padding mask (ColoAttention).

Math (per batch b, head h):
    scores = (Q @ K^T) / sqrt(D); masked keys -> -inf; softmax over keys;
    out = probs @ V; rows at masked query positions zeroed.

Implementation notes:
  - K and V rows at masked key positions are zeroed on the host.  Then
    scores at masked keys are exactly 0, exp(0) = 1, and the per-row sum of
    exponentials just needs the (host-known) masked-key count subtracted.
    Masked keys contribute 0 to probs @ V since their V rows are zero.
  - Scores are computed transposed (S^T[k, q] = K @ Q^T) so the exp output
    P^T[k, q] (bf16) is directly the moving operand for O'^T = V^T @ P^T.
  - Row sums come from a ones-vector matmul over P^T; the softmax division
    and query masking are folded into one per-partition scale applied while
    copying the (PE-transposed) output out of PSUM.
  - QK^T runs in float32r (full-rate fp32 path on the PE), PV in bf16.
  - Sharding: B*H = 64 (b,h) pairs; core c handles batch c//2, heads
    (c%2)*8 .. +8.  Pure SPMD, no collectives.
"""

import numpy as np
import ml_dtypes
from contextlib import ExitStack

import concourse.bass as bass
import concourse.mybir as mybir
import concourse.tile as tile
from concourse import bacc
from concourse.bass_utils import run_bass_kernel_spmd
from concourse.masks import make_identity

B, S, H, D = 4, 2048, 16, 128
N_CORES = 8
CORES_PER_BATCH = N_CORES // B            # 2
HPC = H // CORES_PER_BATCH                # 8 heads (pairs) per core
P = 128
SCALE = 1.0 / float(np.sqrt(np.float64(D)).astype(np.float32))


def build_program(n_pairs: int = HPC, seq: int = S) -> bacc.Bacc:
    KCN = seq // P          # key chunks of 128
    NW = seq // 512         # 512-wide q windows
    QTN = seq // P          # q tiles of 128
    f32 = mybir.dt.float32
    f32r = mybir.dt.float32r
    bf16 = mybir.dt.bfloat16
    Exp = mybir.ActivationFunctionType.Exp

    nc = bacc.Bacc("TRN2", target_bir_lowering=False, debug=False)
    # q/k arrive pre-transposed from the host: [pair, D, S]
    q_d = nc.dram_tensor("q", [n_pairs, P, seq], f32r, kind="ExternalInput").ap()
    k_d = nc.dram_tensor("k", [n_pairs, P, seq], f32r, kind="ExternalInput").ap()
    v_d = nc.dram_tensor("v", [n_pairs, seq, P], bf16, kind="ExternalInput").ap()
    qmask_d = nc.dram_tensor("qmask", [seq], f32, kind="ExternalInput").ap()
    mcnt_d = nc.dram_tensor("mcount", [P, 1], f32, kind="ExternalInput").ap()
    out_d = nc.dram_tensor("out", [n_pairs, seq, P], f32, kind="ExternalOutput").ap()
    sums_d = nc.dram_tensor("sums_scratch", [n_pairs, seq], f32).ap()

    # q is processed in halves of QH columns; scores PSUM double-buffered so
    # QK(kc+1) overlaps exp(kc); ones-matmul row sums interleave per kc.
    QH = min(seq, 1024)      # q-half width
    NH = seq // QH           # number of halves
    NWH = QH // 512          # 512-wide windows per half
    QTH = QH // P            # 128-wide q tiles per half

    with tile.TileContext(nc) as tc:
        with ExitStack() as ctx:
            consts = ctx.enter_context(tc.tile_pool(name="consts", bufs=1))
            inp = ctx.enter_context(tc.tile_pool(name="inp", bufs=2))
            qtp = ctx.enter_context(tc.tile_pool(name="qtp", bufs=2))
            ptp = ctx.enter_context(tc.tile_pool(name="ptp", bufs=2))
            otp = ctx.enter_context(tc.tile_pool(name="otp", bufs=4))
            outp = ctx.enter_context(tc.tile_pool(name="outp", bufs=4))
            smp = ctx.enter_context(tc.tile_pool(name="smp", bufs=2))
            # PSUM (8 banks): "s" scores [128,QH] x2 bufs = 4 banks,
            # "o" O' accum [128,512] x2 = 2 banks (also O transposes),
            # "sm" row sums [1,QH] = 2 banks.
            sps = ctx.enter_context(tc.tile_pool(name="sps", bufs=2, space="PSUM"))
            ops = ctx.enter_context(tc.tile_pool(name="ops", bufs=2, space="PSUM"))
            smps = ctx.enter_context(tc.tile_pool(name="smps", bufs=1, space="PSUM"))

            ident32 = consts.tile([P, P], f32, tag="ident32")
            make_identity(nc, ident32[:])
            ones_b = consts.tile([P, 1], bf16, tag="ones")
            nc.gpsimd.memset(ones_b[:], 1.0)
            qmaskT = consts.tile([P, QTN], f32, tag="qmaskT")
            nc.sync.dma_start(qmaskT[:], qmask_d.rearrange("(t r) -> r t", r=P))
            mcnt = consts.tile([P, 1], f32, tag="mcnt")
            nc.sync.dma_start(mcnt[:], mcnt_d[:])

            for p in range(n_pairs):
                # ---- stage inputs (q/k already transposed on host) ----
                qt_sb = qtp.tile([P, seq], f32r, tag="qt")
                nc.sync.dma_start(qt_sb[:], q_d[p])
                kt_sb = qtp.tile([P, seq], f32r, tag="kt")
                nc.sync.dma_start(kt_sb[:], k_d[p])
                v_sb = inp.tile([P, KCN, P], bf16, tag="v_sb")
                nc.sync.dma_start(v_sb[:], v_d[p].rearrange("(t r) d -> r t d", r=P))

                # ---- scores + exp + PV + row sums, software-pipelined so
                # PE always has QK(step+1) to run while ACT does exp(step).
                def emit_qk(h, kc, tag_i):
                    s_ps = sps.tile([P, QH], f32, tag="s", name=f"s_{p}_{h}_{kc}")
                    for w in range(NWH):
                        nc.tensor.matmul(
                            s_ps[:, w * 512:(w + 1) * 512],
                            lhsT=kt_sb[:, kc * P:(kc + 1) * P],
                            rhs=qt_sb[:, h * QH + w * 512:h * QH + (w + 1) * 512],
                            start=True, stop=True)
                    return s_ps

                steps = [(h, kc) for h in range(NH) for kc in range(KCN)]
                pt_half = {}
                ot_half = {}
                sm_half = {}
                pend = {0: emit_qk(*steps[0], 0)}
                for i, (h, kc) in enumerate(steps):
                    q0 = h * QH
                    if h not in pt_half:
                        pt_half[h] = ptp.tile([P, KCN, QH], bf16, tag="pt",
                                              name=f"pt_{p}_{h}")
                        ot_half[h] = [
                            ops.tile([P, 512], f32, tag="o", name=f"ot_{p}_{h}_{w}")
                            for w in range(NWH)]
                        sm_half[h] = smps.tile([1, QH], f32, tag="sm",
                                               name=f"sm_{p}_{h}")
                    pt_sb, ot_ps, sm_ps = pt_half[h], ot_half[h], sm_half[h]
                    s_ps = pend.pop(i)
                    nc.scalar.activation(pt_sb[:, kc, :], s_ps[:], Exp, scale=SCALE)
                    if i + 1 < len(steps):
                        pend[i + 1] = emit_qk(*steps[i + 1], i + 1)
                    for w in range(NWH):
                        nc.tensor.matmul(
                            ot_ps[w][:],
                            lhsT=v_sb[:, kc, :],
                            rhs=pt_sb[:, kc, w * 512:(w + 1) * 512],
                            start=(kc == 0), stop=(kc == KCN - 1))
                    for w in range(NWH):
                        nc.tensor.matmul(
                            sm_ps[0:1, w * 512:(w + 1) * 512],
                            lhsT=ones_b[:],
                            rhs=pt_sb[:, kc, w * 512:(w + 1) * 512],
                            start=(kc == 0), stop=(kc == KCN - 1))
                    if kc != KCN - 1:
                        continue

                    # ---- half tail: stage O', sums, scale, store ----
                    otsb = []
                    for w in range(NWH):
                        o_stage = otp.tile([P, 512], f32, tag="otsb",
                                           name=f"otsb_{p}_{h}_{w}")
                        nc.vector.tensor_copy(out=o_stage[:], in_=ot_ps[w][:])
                        otsb.append(o_stage)
                    sm_sb = smp.tile([1, QH], f32, tag="sm_sb")
                    nc.vector.tensor_copy(out=sm_sb[:], in_=sm_ps[:])
                    nc.sync.dma_start(sums_d[p, q0:q0 + QH], sm_sb[:])
                    sumsT = smp.tile([P, QTH], f32, tag="sumsT")
                    nc.sync.dma_start(
                        sumsT[:],
                        sums_d[p, q0:q0 + QH].rearrange("(t r) -> r t", r=P))
                    scaleT = smp.tile([P, QTH], f32, tag="scaleT")
                    nc.vector.tensor_scalar_sub(scaleT[:], sumsT[:], mcnt[:, 0:1])
                    nc.vector.reciprocal(scaleT[:], scaleT[:])
                    nc.vector.tensor_tensor(
                        scaleT[:], scaleT[:],
                        qmaskT[:, h * QTH:(h + 1) * QTH], mybir.AluOpType.mult)

                    for w in range(NWH):
                        for j in range(4):
                            qt = w * 4 + j
                            otr = ops.tile([P, P], f32, tag="o")
                            nc.tensor.transpose(
                                otr[:], otsb[w][:, j * P:(j + 1) * P], ident32[:])
                            o_sb = outp.tile([P, P], f32, tag="o_sb")
                            nc.vector.tensor_scalar_mul(
                                o_sb[:], otr[:], scaleT[:, qt:qt + 1])
                            gq = q0 + qt * P
                            nc.sync.dma_start(out_d[p, gq:gq + P, :], o_sb[:])

    nc.compile()
    return nc


_PROG_CACHE: dict = {}


def _get_program() -> bacc.Bacc:
    if "nc" not in _PROG_CACHE:
        _PROG_CACHE["nc"] = build_program(HPC, S)
    return _PROG_CACHE["nc"]


def make_in_maps(query, key, value, attn_mask):
    # device wants q/k as [pair, D, S] (pre-transposed), v as [pair, S, D]
    qT = np.ascontiguousarray(np.asarray(query, np.float32).transpose(0, 2, 3, 1))
    kT = np.asarray(key, np.float32).transpose(0, 2, 3, 1)       # [B, H, D, S]
    v = np.asarray(value, np.float32).transpose(0, 2, 1, 3)      # [B, H, S, D]
    mf = (np.asarray(attn_mask) > 0).astype(np.float32)          # [B, S]
    kTz = np.ascontiguousarray(kT * mf[:, None, None, :])
    vz = (v * mf[:, None, :, None]).astype(ml_dtypes.bfloat16)
    mcount = (S - mf.sum(axis=1)).astype(np.float32)             # [B]
    in_maps = []
    for c in range(N_CORES):
        b, h0 = c // CORES_PER_BATCH, (c % CORES_PER_BATCH) * HPC
        in_maps.append({
            "q": np.ascontiguousarray(qT[b, h0:h0 + HPC]),
            "k": np.ascontiguousarray(kTz[b, h0:h0 + HPC]),
            "v": np.ascontiguousarray(vz[b, h0:h0 + HPC]),
            "qmask": mf[b],
            "mcount": np.full((P, 1), mcount[b], np.float32),
        })
    return in_maps, mf


def assemble_output(results, mf):
    out = np.empty((B, S, H * D), np.float32)
    for c in range(N_CORES):
        b, h0 = c // CORES_PER_BATCH, (c % CORES_PER_BATCH) * HPC
        o = results[c]["out"]                                    # [HPC, S, D]
        for i in range(HPC):
            out[b, :, (h0 + i) * D:(h0 + i + 1) * D] = o[i]
    for b in range(B):
        if mf[b].sum() == 0.0:                                   # degenerate batch
            out[b] = 0.0
    return out


def kernel(query, key, value, attn_mask):
    nc = _get_program()
    in_maps, mf = make_in_maps(query, key, value, attn_mask)
    res = run_bass_kernel_spmd(nc, in_maps, list(range(N_CORES)))
    return assemble_output(res.results, mf)
